# revision 17
# baseline (speedup 1.0000x reference)
"""Multi-head attention (B=4, S=2048, D=512, H=8) on 8 TRN2 NeuronCores.

Sharding: the 8192 query rows (4 batches x 2048 seq) are split into 8 shards
of 1024 rows, one per core (core c -> batch c//2, query-half c%2).  Each core
computes full K/V projections for its batch (duplicated across the pair) and
the full output rows for its queries, so no collective is needed — the host
just concatenates the 8 output shards.

Per-core pipeline (all matmuls bf16, accumulation fp32):
  Q^T  [512,1024] = Wq^T.T @ xq^T   (head-pair-chunk layout, d_k on partitions)
  K^T  [512,2048],  V' [2048, 8*(64+1)]  (V plus a ones column per head ->
                                          softmax denominator for free)
  per head: scores^T[k,q] = K^T.T @ Q^T  (k on partitions ->  mask bias is a
            native per-partition bias of the exp activation; no max-pass)
            P^T = exp(0.125*scores^T + maskbias)      (ScalarE, bf16 out)
            pv[65,1024] += V'_h.T @ P^T               (row 64 = denominator)
            x_attn^T = pv[0:64] * bcast(1/pv[64])     (DVE + gpsimd bcast)
  out[q,e] = x_attn^T.T @ Wo^T + bo  -> DRAM

Heads are processed sequentially through a software-pipelined flat loop
(2-iteration score lookahead) so the scalar engine's exp stream — the
throughput floor of the kernel — stays saturated across head boundaries.
"""
import os
import sys

import numpy as np
import ml_dtypes

try:
    import concourse.bass as bass  # noqa: F401
except ImportError:  # fresh grading dir: fall back to the repo checkout
    for p in ("/root/.axon_site", "/root/.axon_site/_ro/trn_rl_repo",
              "/root/.axon_site/_ro/pypackages", "/opt/trn_rl_repo"):
        if os.path.isdir(p) and p not in sys.path:
            sys.path.insert(0, p)
    import concourse.bass as bass  # noqa: F401

import concourse.mybir as mybir
import concourse.tile as tile
from concourse import bacc
from concourse.bass_utils import run_bass_kernel_spmd

f32 = mybir.dt.float32
bf16 = mybir.dt.bfloat16
fp8 = mybir.dt.float8e4
BF = ml_dtypes.bfloat16
EXPOFF = 1.5          # exp(x - EXPOFF): keeps P < ~64 for fp8e4 range; cancels in softmax

B, S, D, H, DK = 4, 2048, 512, 8, 64
SQ = S // 2          # queries per core
NKT = S // 128       # 16 key tiles
NDC = D // 128       # 4 contraction chunks
PAIRS = H // 2       # 4 head pairs
EXP = mybir.ActivationFunctionType.Exp
MULT = mybir.AluOpType.mult

_NC_CACHE = None


def build_nc():
    global _NC_CACHE
    if _NC_CACHE is not None:
        return _NC_CACHE
    nc = bacc.Bacc("TRN2", target_bir_lowering=False, debug=False, num_devices=8)

    xall = nc.declare_dram_parameter("xall", [NDC, 128, SQ + 2 * S], bf16,
                                     isOutput=False)
    wall = nc.declare_dram_parameter("wall", [NDC, 128, 4 * D], bf16,
                                     isOutput=False)
    ball = nc.declare_dram_parameter("ball", [128, 2 * NDC + NKT], f32,
                                     isOutput=False)
    bvo = nc.declare_dram_parameter("bvo", [1, 2 * D], f32, isOutput=False)
    out = nc.declare_dram_parameter("out", [SQ, D], f32, isOutput=True)

    with tile.TileContext(nc) as tc:
        with (
            tc.tile_pool(name="const", bufs=1) as cp,
            tc.tile_pool(name="xin", bufs=1) as xin,
            tc.tile_pool(name="pt", bufs=8) as ptp,
            tc.tile_pool(name="den", bufs=3) as dnp,
            tc.tile_pool(name="stage", bufs=2) as stp,
            tc.tile_pool(name="rbc", bufs=2) as rbp,
            tc.tile_pool(name="outp", bufs=3) as op,
            tc.tile_pool(name="ps_big", bufs=3, space="PSUM") as ps_big,
            tc.tile_pool(name="ps_pv", bufs=1, space="PSUM") as ps_pv,
        ):
            # ---- constants / weights (fused DMAs; few dma_starts) ----
            wall_sb = cp.tile([128, NDC, 4 * D], bf16, tag="wall")
            xall_sb = cp.tile([128, NDC, SQ + 2 * S], bf16, tag="xall")
            ball_sb = cp.tile([128, 2 * NDC + NKT], f32, tag="ball")
            bvo_sb = cp.tile([1, 2 * D], f32, tag="bvo")
            # issue order = need order: wq, xq, wk, xk, small consts, wv|wo, xv
            nc.sync.dma_start(
                wall_sb[:, :, 0:D],
                wall[:, :, 0:D].rearrange("c p n -> p c n"))
            nc.sync.dma_start(
                xall_sb[:, :, 0:SQ],
                xall[:, :, 0:SQ].rearrange("c p n -> p c n"))
            nc.sync.dma_start(
                wall_sb[:, :, D:2 * D],
                wall[:, :, D:2 * D].rearrange("c p n -> p c n"))
            nc.sync.dma_start(
                xall_sb[:, :, SQ:SQ + S],
                xall[:, :, SQ:SQ + S].rearrange("c p n -> p c n"))
            nc.sync.dma_start(ball_sb[:], ball[:])
            nc.sync.dma_start(bvo_sb[:], bvo[:])
            nc.sync.dma_start(
                wall_sb[:, :, 2 * D:4 * D],
                wall[:, :, 2 * D:4 * D].rearrange("c p n -> p c n"))
            nc.sync.dma_start(
                xall_sb[:, :, SQ + S:SQ + 2 * S],
                xall[:, :, SQ + S:SQ + 2 * S].rearrange("c p n -> p c n"))
            wq_sb = wall_sb[:, :, 0 * D:1 * D]
            wk_sb = wall_sb[:, :, 1 * D:2 * D]
            wv_sb = wall_sb[:, :, 2 * D:3 * D]
            wo_sb = wall_sb[:, :, 3 * D:4 * D]
            bq_sb = ball_sb[:, 0:NDC]
            bk_sb = ball_sb[:, NDC:2 * NDC]
            maskb_sb = ball_sb[:, 2 * NDC:]
            bv_bc = cp.tile([128, D], f32, tag="bv_bc")
            nc.gpsimd.partition_broadcast(bv_bc[:], bvo_sb[0:1, 0:D])
            bo_bc = cp.tile([128, D], f32, tag="bo_bc")
            nc.gpsimd.partition_broadcast(bo_bc[:], bvo_sb[0:1, D:2 * D])

            # ---- persistent activations ----
            QT_sb = cp.tile([128, PAIRS, SQ], bf16, tag="QT")
            KT_sb = cp.tile([128, PAIRS, S], bf16, tag="KT")
            VP_sb = cp.tile([128, NKT, H * 65], bf16, tag="VP")
            XA_sb = cp.tile([128, PAIRS, SQ], bf16, tag="XA")
            # ones column per head inside V'
            vp_ones = VP_sb[:].rearrange("p k (h c) -> p k h c", c=65)[:, :, :, 64:65]
            nc.vector.memset(vp_ones, 1.0)

            xq_c = [xall_sb[:, dc, 0:SQ] for dc in range(NDC)]
            xk_c = [xall_sb[:, dc, SQ:SQ + S] for dc in range(NDC)]
            xv_c = [xall_sb[:, dc, SQ + S:SQ + 2 * S] for dc in range(NDC)]

            # ---- emission helpers ----
            def proj_QT(c, qchs=None):
                for qch in (range(SQ // 512) if qchs is None else qchs):
                    ps = ps_big.tile([128, 1024], f32, tag="big")
                    for dc in range(NDC):
                        nc.tensor.matmul(
                            ps[:, 0:512],
                            wq_sb[:, dc, c * 128:(c + 1) * 128],
                            xq_c[dc][:, qch * 512:(qch + 1) * 512],
                            start=(dc == 0), stop=(dc == NDC - 1),
                        )
                    nc.vector.tensor_scalar_add(
                        QT_sb[:, c, qch * 512:(qch + 1) * 512], ps[:, 0:512],
                        bq_sb[:, c:c + 1],
                    )

            def proj_KT(c, tchs=None):
                for tch in (range(S // 512) if tchs is None else tchs):
                    ps = ps_big.tile([128, 1024], f32, tag="big")
                    for dc in range(NDC):
                        nc.tensor.matmul(
                            ps[:, 0:512],
                            wk_sb[:, dc, c * 128:(c + 1) * 128],
                            xk_c[dc][:, tch * 512:(tch + 1) * 512],
                            start=(dc == 0), stop=(dc == NDC - 1),
                        )
                    nc.vector.tensor_scalar_add(
                        KT_sb[:, c, tch * 512:(tch + 1) * 512], ps[:, 0:512],
                        bk_sb[:, c:c + 1],
                    )

            def proj_V(kt):
                ps = ps_big.tile([128, 1024], f32, tag="big")
                for dc in range(NDC):
                    nc.tensor.matmul(
                        ps[:, 0:512],
                        xv_c[dc][:, kt * 128:(kt + 1) * 128],
                        wv_sb[:, dc, :],
                        start=(dc == 0), stop=(dc == NDC - 1),
                    )
                nc.vector.tensor_tensor(
                    VP_sb[:, kt].rearrange("p (h c) -> p h c", c=65)[:, :, 0:64],
                    ps[:, 0:512].rearrange("p (h c) -> p h c", c=64),
                    bv_bc[:].rearrange("p (h c) -> p h c", c=64),
                    mybir.AluOpType.add,
                )

            def emit_sc(c, half, kt):
                lo, hi = half * 64, (half + 1) * 64
                sc = ps_big.tile([128, 1024], f32, tag="big", name="sc")
                for qch in range(2):
                    nc.tensor.matmul(
                        sc[:, qch * 512:(qch + 1) * 512],
                        KT_sb[lo:hi, c, kt * 128:(kt + 1) * 128],
                        QT_sb[lo:hi, c, qch * 512:(qch + 1) * 512],
                        start=True, stop=True,
                    )
                return sc

            def normalize(c, half, pv, last=False):
                lo, hi = half * 64, (half + 1) * 64
                den = dnp.tile([1, SQ], f32, tag="den")
                nc.vector.tensor_copy(den[:], pv[64:65, 0:SQ])
                rec = dnp.tile([1, SQ], f32, tag="rec")
                nc.vector.reciprocal_approx_fast(out=rec[:], in_=den[:])
                rbc = rbp.tile([64, SQ], f32, tag="rbc")
                nc.gpsimd.partition_broadcast(rbc[:], rec[:])
                if last:
                    # tail head: multiply straight out of PSUM, skip staging
                    nc.vector.tensor_tensor(
                        XA_sb[lo:hi, c, :], pv[0:64, 0:SQ], rbc[:], MULT,
                    )
                else:
                    stg = stp.tile([64, SQ], f32, tag="stg")
                    nc.vector.tensor_copy(stg[:], pv[0:64, 0:SQ])
                    nc.vector.tensor_tensor(
                        XA_sb[lo:hi, c, :], stg[:], rbc[:], MULT,
                    )

            def attention_all():
                its = [(c, half, kt)
                       for c in range(PAIRS) for half in range(2)
                       for kt in range(NKT)]
                sc_t = {}
                sc_t[its[0]] = emit_sc(*its[0])
                sc_t[its[1]] = emit_sc(*its[1])
                pv = None
                for i, (c, half, kt) in enumerate(its):
                    h = 2 * c + half
                    if kt == 0:
                        pv = ps_pv.tile([128, 1024], f32, tag="pv",
                                        name=f"pv{h}")
                    if i + 2 < len(its):
                        sc_t[its[i + 2]] = emit_sc(*its[i + 2])
                    if h == 0:
                        proj_V(kt)
                    if half == 1 and 8 <= kt < 12 and c < PAIRS - 1:
                        proj_KT(c + 1, [kt - 8])
                    sc = sc_t.pop((c, half, kt))
                    pt = ptp.tile([128, 1024], bf16, tag="pt")
                    nc.scalar.activation(
                        pt[:], sc[:], EXP,
                        bias=maskb_sb[:, kt:kt + 1], scale=0.125,
                    )
                    for qch in range(2):
                        nc.tensor.matmul(
                            pv[0:65, qch * 512:(qch + 1) * 512],
                            VP_sb[:, kt, h * 65:(h + 1) * 65],
                            pt[:, qch * 512:(qch + 1) * 512],
                            start=(kt == 0), stop=(kt == NKT - 1),
                        )
                    if kt == NKT - 1:
                        normalize(c, half, pv, last=(h == H - 1))

            def wo_out(qt):
                ps = ps_big.tile([128, 1024], f32, tag="big")
                for j in range(NDC):
                    nc.tensor.matmul(
                        ps[:, 0:512],
                        XA_sb[:, j, qt * 128:(qt + 1) * 128],
                        wo_sb[:, j, :],
                        start=(j == 0), stop=(j == NDC - 1),
                    )
                nc.vector.tensor_tensor(
                    ob_all[:, qt, :], ps[:, 0:512], bo_bc[:],
                    mybir.AluOpType.add,
                )

            # ---- schedule ----
            for c in range(PAIRS):
                proj_QT(c)
            proj_KT(0)
            attention_all()
            ob_all = cp.tile([128, SQ // 128, D], f32, tag="ob_all")
            outr = out[:].rearrange("(q p) d -> p q d", p=128)
            for qt in range(SQ // 128):
                wo_out(qt)
                if qt == 3:
                    nc.sync.dma_start(outr[:, 0:4], ob_all[:, 0:4])
            nc.sync.dma_start(outr[:, 4:8], ob_all[:, 4:8])

    nc.finalize()
    _NC_CACHE = nc
    return nc


def make_in_maps(query, key, value, mask, Wq, bq, Wk, bk, Wv, bv, Wo, bo):
    query = np.asarray(query, np.float32)
    key = np.asarray(key, np.float32)
    value = np.asarray(value, np.float32)
    mask = np.asarray(mask)

    def wprep(W):
        return np.ascontiguousarray(
            np.asarray(W, np.float32).T.reshape(NDC, 128, D)
        ).astype(BF)

    wall_a = np.ascontiguousarray(np.concatenate(
        [wprep(Wq), wprep(Wk), wprep(Wv), wprep(Wo)], axis=2))
    bq_a = np.asarray(bq, np.float32).reshape(NDC, 128).T
    bk_a = np.asarray(bk, np.float32).reshape(NDC, 128).T
    bvo_a = np.ascontiguousarray(np.concatenate(
        [np.asarray(bv, np.float32).reshape(1, D),
         np.asarray(bo, np.float32).reshape(1, D)], axis=1))

    kT = key.transpose(0, 2, 1)    # [B, D, S]
    vT = value.transpose(0, 2, 1)
    qT = query.transpose(0, 2, 1)

    in_maps = []
    for core in range(8):
        b, qh = core // 2, core % 2
        xq_a = qT[b][:, qh * SQ:(qh + 1) * SQ].reshape(NDC, 128, SQ)
        xk_a = kT[b].reshape(NDC, 128, S)
        xv_a = vT[b].reshape(NDC, 128, S)
        xall_a = np.ascontiguousarray(
            np.concatenate([xq_a, xk_a, xv_a], axis=2)).astype(BF)
        mb = np.where(mask[b, 0] == 0, np.float32(-1e9), np.float32(0.0))
        mb = mb.reshape(NKT, 128).T
        ball_a = np.ascontiguousarray(
            np.concatenate([bq_a, bk_a, mb], axis=1)).astype(np.float32)
        in_maps.append({
            "xall": xall_a, "wall": wall_a, "ball": ball_a, "bvo": bvo_a,
        })
    return in_maps


def assemble_output(results):
    full = np.empty((B, S, D), np.float32)
    for core in range(8):
        b, qh = core // 2, core % 2
        full[b, qh * SQ:(qh + 1) * SQ, :] = results[core]["out"]
    return full


def kernel(**inputs):
    nc = build_nc()
    in_maps = make_in_maps(**inputs)
    res = run_bass_kernel_spmd(nc, in_maps, list(range(8))).results
    return assemble_output(res)


# revision 18
# speedup vs baseline: 1.0191x; 1.0191x over previous
"""Multi-head attention (B=4, S=2048, D=512, H=8) on 8 TRN2 NeuronCores.

Sharding: the 8192 query rows (4 batches x 2048 seq) are split into 8 shards
of 1024 rows, one per core (core c -> batch c//2, query-half c%2).  Each core
computes full K/V projections for its batch (duplicated across the pair) and
the full output rows for its queries, so no collective is needed — the host
just concatenates the 8 output shards.

Per-core pipeline (all matmuls bf16, accumulation fp32):
  Q^T  [512,1024] = Wq^T.T @ xq^T   (head-pair-chunk layout, d_k on partitions)
  K^T  [512,2048],  V' [2048, 8*(64+1)]  (V plus a ones column per head ->
                                          softmax denominator for free)
  per head: scores^T[k,q] = K^T.T @ Q^T  (k on partitions ->  mask bias is a
            native per-partition bias of the exp activation; no max-pass)
            P^T = exp(0.125*scores^T + maskbias)      (ScalarE, bf16 out)
            pv[65,1024] += V'_h.T @ P^T               (row 64 = denominator)
            x_attn^T = pv[0:64] * bcast(1/pv[64])     (DVE + gpsimd bcast)
  out[q,e] = x_attn^T.T @ Wo^T + bo  -> DRAM

Heads are processed sequentially through a software-pipelined flat loop
(2-iteration score lookahead) so the scalar engine's exp stream — the
throughput floor of the kernel — stays saturated across head boundaries.
"""
import os
import sys

import numpy as np
import ml_dtypes

try:
    import concourse.bass as bass  # noqa: F401
except ImportError:  # fresh grading dir: fall back to the repo checkout
    for p in ("/root/.axon_site", "/root/.axon_site/_ro/trn_rl_repo",
              "/root/.axon_site/_ro/pypackages", "/opt/trn_rl_repo"):
        if os.path.isdir(p) and p not in sys.path:
            sys.path.insert(0, p)
    import concourse.bass as bass  # noqa: F401

import concourse.mybir as mybir
import concourse.tile as tile
from concourse import bacc
from concourse.bass_utils import run_bass_kernel_spmd

f32 = mybir.dt.float32
bf16 = mybir.dt.bfloat16
fp8 = mybir.dt.float8e4
BF = ml_dtypes.bfloat16
EXPOFF = 1.5          # exp(x - EXPOFF): keeps P < ~64 for fp8e4 range; cancels in softmax

B, S, D, H, DK = 4, 2048, 512, 8, 64
SQ = S // 2          # queries per core
NKT = S // 128       # 16 key tiles
NDC = D // 128       # 4 contraction chunks
PAIRS = H // 2       # 4 head pairs
EXP = mybir.ActivationFunctionType.Exp
MULT = mybir.AluOpType.mult

_NC_CACHE = None


def build_nc():
    global _NC_CACHE
    if _NC_CACHE is not None:
        return _NC_CACHE
    nc = bacc.Bacc("TRN2", target_bir_lowering=False, debug=False, num_devices=8)

    xall = nc.declare_dram_parameter("xall", [NDC, 128, SQ + 2 * S], bf16,
                                     isOutput=False)
    wall = nc.declare_dram_parameter("wall", [NDC, 128, 4 * D], bf16,
                                     isOutput=False)
    ball = nc.declare_dram_parameter("ball", [128, 2 * NDC + NKT], f32,
                                     isOutput=False)
    bvo = nc.declare_dram_parameter("bvo", [1, 2 * D], f32, isOutput=False)
    out = nc.declare_dram_parameter("out", [SQ, D], f32, isOutput=True)

    with tile.TileContext(nc) as tc:
        with (
            tc.tile_pool(name="const", bufs=1) as cp,
            tc.tile_pool(name="xin", bufs=1) as xin,
            tc.tile_pool(name="pt", bufs=6) as ptp,
            tc.tile_pool(name="den", bufs=3) as dnp,
            tc.tile_pool(name="stage", bufs=2) as stp,
            tc.tile_pool(name="rbc", bufs=2) as rbp,
            tc.tile_pool(name="outp", bufs=3) as op,
            tc.tile_pool(name="ps_big", bufs=3, space="PSUM") as ps_big,
            tc.tile_pool(name="ps_pv", bufs=1, space="PSUM") as ps_pv,
        ):
            # ---- constants / weights (fused DMAs; few dma_starts) ----
            wall_sb = cp.tile([128, NDC, 4 * D], bf16, tag="wall")
            xall_sb = cp.tile([128, NDC, SQ + 2 * S], bf16, tag="xall")
            ball_sb = cp.tile([128, 2 * NDC + NKT], f32, tag="ball")
            bvo_sb = cp.tile([1, 2 * D], f32, tag="bvo")
            # issue order = need order: wq, xq, wk, xk, small consts, wv|wo, xv
            nc.sync.dma_start(
                wall_sb[:, :, 0:D],
                wall[:, :, 0:D].rearrange("c p n -> p c n"))
            nc.sync.dma_start(
                xall_sb[:, :, 0:SQ],
                xall[:, :, 0:SQ].rearrange("c p n -> p c n"))
            nc.sync.dma_start(
                wall_sb[:, :, D:2 * D],
                wall[:, :, D:2 * D].rearrange("c p n -> p c n"))
            nc.sync.dma_start(
                xall_sb[:, :, SQ:SQ + S],
                xall[:, :, SQ:SQ + S].rearrange("c p n -> p c n"))
            nc.sync.dma_start(ball_sb[:], ball[:])
            nc.sync.dma_start(bvo_sb[:], bvo[:])
            nc.sync.dma_start(
                wall_sb[:, :, 2 * D:4 * D],
                wall[:, :, 2 * D:4 * D].rearrange("c p n -> p c n"))
            nc.sync.dma_start(
                xall_sb[:, :, SQ + S:SQ + 2 * S],
                xall[:, :, SQ + S:SQ + 2 * S].rearrange("c p n -> p c n"))
            wq_sb = wall_sb[:, :, 0 * D:1 * D]
            wk_sb = wall_sb[:, :, 1 * D:2 * D]
            wv_sb = wall_sb[:, :, 2 * D:3 * D]
            wo_sb = wall_sb[:, :, 3 * D:4 * D]
            bq_sb = ball_sb[:, 0:NDC]
            bk_sb = ball_sb[:, NDC:2 * NDC]
            maskb_sb = ball_sb[:, 2 * NDC:]
            bv_bc = cp.tile([128, D], f32, tag="bv_bc")
            nc.gpsimd.partition_broadcast(bv_bc[:], bvo_sb[0:1, 0:D])
            bo_bc = cp.tile([128, D], f32, tag="bo_bc")
            nc.gpsimd.partition_broadcast(bo_bc[:], bvo_sb[0:1, D:2 * D])

            # ---- persistent activations ----
            QT_sb = cp.tile([128, PAIRS, SQ], bf16, tag="QT")
            KT_sb = cp.tile([128, PAIRS, S], bf16, tag="KT")
            VP_sb = cp.tile([128, NKT, H * 65], bf16, tag="VP")
            XA_sb = cp.tile([128, PAIRS, SQ], bf16, tag="XA")
            # ones column per head inside V'
            vp_ones = VP_sb[:].rearrange("p k (h c) -> p k h c", c=65)[:, :, :, 64:65]
            nc.vector.memset(vp_ones, 1.0)

            xq_c = [xall_sb[:, dc, 0:SQ] for dc in range(NDC)]
            xk_c = [xall_sb[:, dc, SQ:SQ + S] for dc in range(NDC)]
            xv_c = [xall_sb[:, dc, SQ + S:SQ + 2 * S] for dc in range(NDC)]

            # ---- emission helpers ----
            def proj_QT(c, qchs=None):
                for qch in (range(SQ // 512) if qchs is None else qchs):
                    ps = ps_big.tile([128, 1024], f32, tag="big")
                    for dc in range(NDC):
                        nc.tensor.matmul(
                            ps[:, 0:512],
                            wq_sb[:, dc, c * 128:(c + 1) * 128],
                            xq_c[dc][:, qch * 512:(qch + 1) * 512],
                            start=(dc == 0), stop=(dc == NDC - 1),
                        )
                    nc.vector.tensor_scalar_add(
                        QT_sb[:, c, qch * 512:(qch + 1) * 512], ps[:, 0:512],
                        bq_sb[:, c:c + 1],
                    )

            def proj_KT(c, tchs=None):
                for tch in (range(S // 512) if tchs is None else tchs):
                    ps = ps_big.tile([128, 1024], f32, tag="big")
                    for dc in range(NDC):
                        nc.tensor.matmul(
                            ps[:, 0:512],
                            wk_sb[:, dc, c * 128:(c + 1) * 128],
                            xk_c[dc][:, tch * 512:(tch + 1) * 512],
                            start=(dc == 0), stop=(dc == NDC - 1),
                        )
                    nc.vector.tensor_scalar_add(
                        KT_sb[:, c, tch * 512:(tch + 1) * 512], ps[:, 0:512],
                        bk_sb[:, c:c + 1],
                    )

            def proj_V(kt):
                ps = ps_big.tile([128, 1024], f32, tag="big")
                for dc in range(NDC):
                    nc.tensor.matmul(
                        ps[:, 0:512],
                        xv_c[dc][:, kt * 128:(kt + 1) * 128],
                        wv_sb[:, dc, :],
                        start=(dc == 0), stop=(dc == NDC - 1),
                    )
                nc.vector.tensor_tensor(
                    VP_sb[:, kt].rearrange("p (h c) -> p h c", c=65)[:, :, 0:64],
                    ps[:, 0:512].rearrange("p (h c) -> p h c", c=64),
                    bv_bc[:].rearrange("p (h c) -> p h c", c=64),
                    mybir.AluOpType.add,
                )

            def emit_sc(c, half, kt):
                lo, hi = half * 64, (half + 1) * 64
                sc = ps_big.tile([128, 1024], f32, tag="big", name="sc")
                for qch in range(2):
                    nc.tensor.matmul(
                        sc[:, qch * 512:(qch + 1) * 512],
                        KT_sb[lo:hi, c, kt * 128:(kt + 1) * 128],
                        QT_sb[lo:hi, c, qch * 512:(qch + 1) * 512],
                        start=True, stop=True,
                    )
                return sc

            def normalize(c, half, pv, last=False):
                lo, hi = half * 64, (half + 1) * 64
                den = dnp.tile([1, SQ], f32, tag="den")
                nc.vector.tensor_copy(den[:], pv[64:65, 0:SQ])
                rec = dnp.tile([1, SQ], f32, tag="rec")
                nc.vector.reciprocal_approx_fast(out=rec[:], in_=den[:])
                rbc = rbp.tile([64, SQ], f32, tag="rbc")
                nc.gpsimd.partition_broadcast(rbc[:], rec[:])
                if last:
                    # tail head: multiply straight out of PSUM, skip staging
                    nc.vector.tensor_tensor(
                        XA_sb[lo:hi, c, :], pv[0:64, 0:SQ], rbc[:], MULT,
                    )
                else:
                    stg = stp.tile([64, SQ], f32, tag="stg")
                    nc.vector.tensor_copy(stg[:], pv[0:64, 0:SQ])
                    nc.vector.tensor_tensor(
                        XA_sb[lo:hi, c, :], stg[:], rbc[:], MULT,
                    )

            def attention_all():
                its = [(c, half, kt)
                       for c in range(PAIRS) for half in range(2)
                       for kt in range(NKT)]
                sc_t = {}
                sc_t[its[0]] = emit_sc(*its[0])
                sc_t[its[1]] = emit_sc(*its[1])
                pv = None
                for i, (c, half, kt) in enumerate(its):
                    h = 2 * c + half
                    if kt == 0:
                        pv = ps_pv.tile([128, 1024], f32, tag="pv",
                                        name=f"pv{h}")
                    if i + 2 < len(its):
                        sc_t[its[i + 2]] = emit_sc(*its[i + 2])
                    if h == 0:
                        proj_V(kt)
                    if half == 1 and 8 <= kt < 12 and c < PAIRS - 1:
                        proj_KT(c + 1, [kt - 8])
                    sc = sc_t.pop((c, half, kt))
                    pt = ptp.tile([128, 1024], bf16, tag="pt")
                    nc.scalar.activation(
                        pt[:], sc[:], EXP,
                        bias=maskb_sb[:, kt:kt + 1], scale=0.125,
                    )
                    for qch in range(2):
                        nc.tensor.matmul(
                            pv[0:65, qch * 512:(qch + 1) * 512],
                            VP_sb[:, kt, h * 65:(h + 1) * 65],
                            pt[:, qch * 512:(qch + 1) * 512],
                            start=(kt == 0), stop=(kt == NKT - 1),
                        )
                    if kt == NKT - 1:
                        normalize(c, half, pv, last=(h == H - 1))

            def wo_out(qt):
                ps = ps_big.tile([128, 1024], f32, tag="big")
                for j in range(NDC):
                    nc.tensor.matmul(
                        ps[:, 0:512],
                        XA_sb[:, j, qt * 128:(qt + 1) * 128],
                        wo_sb[:, j, :],
                        start=(j == 0), stop=(j == NDC - 1),
                    )
                nc.vector.tensor_tensor(
                    ob_all[:, qt, :], ps[:, 0:512], bo_bc[:],
                    mybir.AluOpType.add,
                )

            # ---- schedule ----
            for c in range(PAIRS):
                proj_QT(c)
            proj_KT(0)
            attention_all()
            ob_all = cp.tile([128, SQ // 128, D], f32, tag="ob_all")
            outr = out[:].rearrange("(q p) d -> p q d", p=128)
            for qt in range(SQ // 128):
                wo_out(qt)
                if qt == 3:
                    nc.sync.dma_start(outr[:, 0:4], ob_all[:, 0:4])
            nc.sync.dma_start(outr[:, 4:8], ob_all[:, 4:8])

    nc.finalize()
    _NC_CACHE = nc
    return nc


def make_in_maps(query, key, value, mask, Wq, bq, Wk, bk, Wv, bv, Wo, bo):
    query = np.asarray(query, np.float32)
    key = np.asarray(key, np.float32)
    value = np.asarray(value, np.float32)
    mask = np.asarray(mask)

    def wprep(W):
        return np.ascontiguousarray(
            np.asarray(W, np.float32).T.reshape(NDC, 128, D)
        ).astype(BF)

    wall_a = np.ascontiguousarray(np.concatenate(
        [wprep(Wq), wprep(Wk), wprep(Wv), wprep(Wo)], axis=2))
    bq_a = np.asarray(bq, np.float32).reshape(NDC, 128).T
    bk_a = np.asarray(bk, np.float32).reshape(NDC, 128).T
    bvo_a = np.ascontiguousarray(np.concatenate(
        [np.asarray(bv, np.float32).reshape(1, D),
         np.asarray(bo, np.float32).reshape(1, D)], axis=1))

    kT = key.transpose(0, 2, 1)    # [B, D, S]
    vT = value.transpose(0, 2, 1)
    qT = query.transpose(0, 2, 1)

    in_maps = []
    for core in range(8):
        b, qh = core // 2, core % 2
        xq_a = qT[b][:, qh * SQ:(qh + 1) * SQ].reshape(NDC, 128, SQ)
        xk_a = kT[b].reshape(NDC, 128, S)
        xv_a = vT[b].reshape(NDC, 128, S)
        xall_a = np.ascontiguousarray(
            np.concatenate([xq_a, xk_a, xv_a], axis=2)).astype(BF)
        mb = np.where(mask[b, 0] == 0, np.float32(-1e9), np.float32(0.0))
        mb = mb.reshape(NKT, 128).T
        ball_a = np.ascontiguousarray(
            np.concatenate([bq_a, bk_a, mb], axis=1)).astype(np.float32)
        in_maps.append({
            "xall": xall_a, "wall": wall_a, "ball": ball_a, "bvo": bvo_a,
        })
    return in_maps


def assemble_output(results):
    full = np.empty((B, S, D), np.float32)
    for core in range(8):
        b, qh = core // 2, core % 2
        full[b, qh * SQ:(qh + 1) * SQ, :] = results[core]["out"]
    return full


def kernel(**inputs):
    nc = build_nc()
    in_maps = make_in_maps(**inputs)
    res = run_bass_kernel_spmd(nc, in_maps, list(range(8))).results
    return assemble_output(res)


# revision 19
# speedup vs baseline: 1.0416x; 1.0221x over previous
"""Multi-head attention (B=4, S=2048, D=512, H=8) on 8 TRN2 NeuronCores.

Sharding: the 8192 query rows (4 batches x 2048 seq) are split into 8 shards
of 1024 rows, one per core (core c -> batch c//2, query-half c%2).  Each core
computes full K/V projections for its batch (duplicated across the pair) and
the full output rows for its queries, so no collective is needed — the host
just concatenates the 8 output shards.

Per-core pipeline (all matmuls bf16, accumulation fp32):
  Q^T  [512,1024] = Wq^T.T @ xq^T   (head-pair-chunk layout, d_k on partitions)
  K^T  [512,2048],  V' [2048, 8*(64+1)]  (V plus a ones column per head ->
                                          softmax denominator for free)
  per head: scores^T[k,q] = K^T.T @ Q^T  (k on partitions ->  mask bias is a
            native per-partition bias of the exp activation; no max-pass)
            P^T = exp(0.125*scores^T + maskbias)      (ScalarE, bf16 out)
            pv[65,1024] += V'_h.T @ P^T               (row 64 = denominator)
            x_attn^T = pv[0:64] * bcast(1/pv[64])     (DVE + gpsimd bcast)
  out[q,e] = x_attn^T.T @ Wo^T + bo  -> DRAM

Heads are processed sequentially through a software-pipelined flat loop
(2-iteration score lookahead) so the scalar engine's exp stream — the
throughput floor of the kernel — stays saturated across head boundaries.
"""
import os
import sys

import numpy as np
import ml_dtypes

try:
    import concourse.bass as bass  # noqa: F401
except ImportError:  # fresh grading dir: fall back to the repo checkout
    for p in ("/root/.axon_site", "/root/.axon_site/_ro/trn_rl_repo",
              "/root/.axon_site/_ro/pypackages", "/opt/trn_rl_repo"):
        if os.path.isdir(p) and p not in sys.path:
            sys.path.insert(0, p)
    import concourse.bass as bass  # noqa: F401

import concourse.mybir as mybir
import concourse.tile as tile
from concourse import bacc
from concourse.bass_utils import run_bass_kernel_spmd

f32 = mybir.dt.float32
bf16 = mybir.dt.bfloat16
fp8 = mybir.dt.float8e4
BF = ml_dtypes.bfloat16
EXPOFF = 1.5          # exp(x - EXPOFF): keeps P < ~64 for fp8e4 range; cancels in softmax

B, S, D, H, DK = 4, 2048, 512, 8, 64
SQ = S // 2          # queries per core
NKT = S // 128       # 16 key tiles
NDC = D // 128       # 4 contraction chunks
PAIRS = H // 2       # 4 head pairs
EXP = mybir.ActivationFunctionType.Exp
MULT = mybir.AluOpType.mult

_NC_CACHE = None


def build_nc():
    global _NC_CACHE
    if _NC_CACHE is not None:
        return _NC_CACHE
    nc = bacc.Bacc("TRN2", target_bir_lowering=False, debug=False, num_devices=8)

    xall = nc.declare_dram_parameter("xall", [NDC, 128, SQ + 2 * S], bf16,
                                     isOutput=False)
    wall = nc.declare_dram_parameter("wall", [NDC, 128, 4 * D], bf16,
                                     isOutput=False)
    ball = nc.declare_dram_parameter("ball", [128, 2 * NDC + NKT], f32,
                                     isOutput=False)
    bvo = nc.declare_dram_parameter("bvo", [1, 2 * D], f32, isOutput=False)
    out = nc.declare_dram_parameter("out", [SQ, D], f32, isOutput=True)

    with tile.TileContext(nc) as tc:
        with (
            tc.tile_pool(name="const", bufs=1) as cp,
            tc.tile_pool(name="xin", bufs=1) as xin,
            tc.tile_pool(name="pt", bufs=6) as ptp,
            tc.tile_pool(name="den", bufs=3) as dnp,
            tc.tile_pool(name="stage", bufs=2) as stp,
            tc.tile_pool(name="rbc", bufs=2) as rbp,
            tc.tile_pool(name="outp", bufs=3) as op,
            tc.tile_pool(name="ps_big", bufs=3, space="PSUM") as ps_big,
            tc.tile_pool(name="ps_pv", bufs=1, space="PSUM") as ps_pv,
        ):
            # ---- constants / weights (fused DMAs; few dma_starts) ----
            wall_sb = cp.tile([128, NDC, 4 * D], bf16, tag="wall")
            xall_sb = cp.tile([128, NDC, SQ + 2 * S], bf16, tag="xall")
            ball_sb = cp.tile([128, 2 * NDC + NKT], f32, tag="ball")
            bvo_sb = cp.tile([1, 2 * D], f32, tag="bvo")
            # issue order = need order: wq, xq, wk, xk, small consts, wv|wo, xv
            nc.sync.dma_start(
                wall_sb[:, :, 0:D],
                wall[:, :, 0:D].rearrange("c p n -> p c n"))
            nc.sync.dma_start(
                xall_sb[:, :, 0:SQ],
                xall[:, :, 0:SQ].rearrange("c p n -> p c n"))
            nc.sync.dma_start(
                wall_sb[:, :, D:2 * D],
                wall[:, :, D:2 * D].rearrange("c p n -> p c n"))
            nc.sync.dma_start(
                xall_sb[:, :, SQ:SQ + S],
                xall[:, :, SQ:SQ + S].rearrange("c p n -> p c n"))
            nc.sync.dma_start(ball_sb[:], ball[:])
            nc.sync.dma_start(bvo_sb[:], bvo[:])
            nc.sync.dma_start(
                wall_sb[:, :, 2 * D:4 * D],
                wall[:, :, 2 * D:4 * D].rearrange("c p n -> p c n"))
            nc.sync.dma_start(
                xall_sb[:, :, SQ + S:SQ + 2 * S],
                xall[:, :, SQ + S:SQ + 2 * S].rearrange("c p n -> p c n"))
            wq_sb = wall_sb[:, :, 0 * D:1 * D]
            wk_sb = wall_sb[:, :, 1 * D:2 * D]
            wv_sb = wall_sb[:, :, 2 * D:3 * D]
            wo_sb = wall_sb[:, :, 3 * D:4 * D]
            bq_sb = ball_sb[:, 0:NDC]
            bk_sb = ball_sb[:, NDC:2 * NDC]
            maskb_sb = ball_sb[:, 2 * NDC:]
            bv_bc = cp.tile([128, D], f32, tag="bv_bc")
            nc.gpsimd.partition_broadcast(bv_bc[:], bvo_sb[0:1, 0:D])
            bo_bc = cp.tile([128, D], f32, tag="bo_bc")
            nc.gpsimd.partition_broadcast(bo_bc[:], bvo_sb[0:1, D:2 * D])

            # ---- persistent activations ----
            QT_sb = cp.tile([128, PAIRS, SQ], bf16, tag="QT")
            KT_sb = cp.tile([128, PAIRS, S], bf16, tag="KT")
            VP_sb = cp.tile([128, NKT, H * 65], bf16, tag="VP")
            XA_sb = cp.tile([128, PAIRS, SQ], bf16, tag="XA")
            # ones column per head inside V'
            vp_ones = VP_sb[:].rearrange("p k (h c) -> p k h c", c=65)[:, :, :, 64:65]
            nc.vector.memset(vp_ones, 1.0)

            xq_c = [xall_sb[:, dc, 0:SQ] for dc in range(NDC)]
            xk_c = [xall_sb[:, dc, SQ:SQ + S] for dc in range(NDC)]
            xv_c = [xall_sb[:, dc, SQ + S:SQ + 2 * S] for dc in range(NDC)]

            # ---- emission helpers ----
            def proj_QT(c, qchs=None):
                for qch in (range(SQ // 512) if qchs is None else qchs):
                    ps = ps_big.tile([128, 1024], f32, tag="big")
                    for dc in range(NDC):
                        nc.tensor.matmul(
                            ps[:, 0:512],
                            wq_sb[:, dc, c * 128:(c + 1) * 128],
                            xq_c[dc][:, qch * 512:(qch + 1) * 512],
                            start=(dc == 0), stop=(dc == NDC - 1),
                        )
                    nc.vector.tensor_scalar_add(
                        QT_sb[:, c, qch * 512:(qch + 1) * 512], ps[:, 0:512],
                        bq_sb[:, c:c + 1],
                    )

            def proj_KT(c, tchs=None):
                for tch in (range(S // 512) if tchs is None else tchs):
                    ps = ps_big.tile([128, 1024], f32, tag="big")
                    for dc in range(NDC):
                        nc.tensor.matmul(
                            ps[:, 0:512],
                            wk_sb[:, dc, c * 128:(c + 1) * 128],
                            xk_c[dc][:, tch * 512:(tch + 1) * 512],
                            start=(dc == 0), stop=(dc == NDC - 1),
                        )
                    nc.vector.tensor_scalar_add(
                        KT_sb[:, c, tch * 512:(tch + 1) * 512], ps[:, 0:512],
                        bk_sb[:, c:c + 1],
                    )

            def proj_V(kt):
                ps = ps_big.tile([128, 1024], f32, tag="big")
                for dc in range(NDC):
                    nc.tensor.matmul(
                        ps[:, 0:512],
                        xv_c[dc][:, kt * 128:(kt + 1) * 128],
                        wv_sb[:, dc, :],
                        start=(dc == 0), stop=(dc == NDC - 1),
                    )
                nc.vector.tensor_tensor(
                    VP_sb[:, kt].rearrange("p (h c) -> p h c", c=65)[:, :, 0:64],
                    ps[:, 0:512].rearrange("p (h c) -> p h c", c=64),
                    bv_bc[:].rearrange("p (h c) -> p h c", c=64),
                    mybir.AluOpType.add,
                )

            def emit_sc(c, half, kt):
                lo, hi = half * 64, (half + 1) * 64
                sc = ps_big.tile([128, 1024], f32, tag="big", name="sc")
                for qch in range(2):
                    nc.tensor.matmul(
                        sc[:, qch * 512:(qch + 1) * 512],
                        KT_sb[lo:hi, c, kt * 128:(kt + 1) * 128],
                        QT_sb[lo:hi, c, qch * 512:(qch + 1) * 512],
                        start=True, stop=True,
                    )
                return sc

            def normalize(c, half, pv):
                lo, hi = half * 64, (half + 1) * 64
                den = dnp.tile([1, SQ], f32, tag="den")
                nc.vector.tensor_copy(den[:], pv[64:65, 0:SQ])
                stg = stp.tile([64, SQ], f32, tag="stg")
                nc.vector.tensor_copy(stg[:], pv[0:64, 0:SQ])
                rec = dnp.tile([1, SQ], f32, tag="rec")
                nc.vector.reciprocal_approx_fast(out=rec[:], in_=den[:])
                rbc = rbp.tile([64, SQ], f32, tag="rbc")
                nc.gpsimd.partition_broadcast(rbc[:], rec[:])
                nc.vector.tensor_tensor(
                    XA_sb[lo:hi, c, :], stg[:], rbc[:], MULT,
                )

            def attention_all():
                its = [(c, half, kt)
                       for c in range(PAIRS) for half in range(2)
                       for kt in range(NKT)]
                sc_t = {}
                sc_t[its[0]] = emit_sc(*its[0])
                sc_t[its[1]] = emit_sc(*its[1])
                pv = None
                for i, (c, half, kt) in enumerate(its):
                    h = 2 * c + half
                    if kt == 0:
                        pv = ps_pv.tile([128, 1024], f32, tag="pv",
                                        name=f"pv{h}")
                    if i + 2 < len(its):
                        sc_t[its[i + 2]] = emit_sc(*its[i + 2])
                    if h == 0:
                        proj_V(kt)
                    if half == 1 and 8 <= kt < 12 and c < PAIRS - 1:
                        proj_KT(c + 1, [kt - 8])
                    sc = sc_t.pop((c, half, kt))
                    pt = ptp.tile([128, 1024], bf16, tag="pt")
                    nc.scalar.activation(
                        pt[:], sc[:], EXP,
                        bias=maskb_sb[:, kt:kt + 1], scale=0.125,
                    )
                    for qch in range(2):
                        nc.tensor.matmul(
                            pv[0:65, qch * 512:(qch + 1) * 512],
                            VP_sb[:, kt, h * 65:(h + 1) * 65],
                            pt[:, qch * 512:(qch + 1) * 512],
                            start=(kt == 0), stop=(kt == NKT - 1),
                        )
                    if kt == NKT - 1:
                        normalize(c, half, pv)

            def wo_out(qt):
                ps = ps_big.tile([128, 1024], f32, tag="big")
                for j in range(NDC):
                    nc.tensor.matmul(
                        ps[:, 0:512],
                        XA_sb[:, j, qt * 128:(qt + 1) * 128],
                        wo_sb[:, j, :],
                        start=(j == 0), stop=(j == NDC - 1),
                    )
                nc.vector.tensor_tensor(
                    ob_all[:, qt, :], ps[:, 0:512], bo_bc[:],
                    mybir.AluOpType.add,
                )

            # ---- schedule ----
            for c in range(PAIRS):
                proj_QT(c)
            proj_KT(0)
            attention_all()
            ob_all = cp.tile([128, SQ // 128, D], f32, tag="ob_all")
            outr = out[:].rearrange("(q p) d -> p q d", p=128)
            for qt in range(SQ // 128):
                wo_out(qt)
                if qt == 3:
                    nc.sync.dma_start(outr[:, 0:4], ob_all[:, 0:4])
            nc.sync.dma_start(outr[:, 4:8], ob_all[:, 4:8])

    nc.finalize()
    _NC_CACHE = nc
    return nc


def make_in_maps(query, key, value, mask, Wq, bq, Wk, bk, Wv, bv, Wo, bo):
    query = np.asarray(query, np.float32)
    key = np.asarray(key, np.float32)
    value = np.asarray(value, np.float32)
    mask = np.asarray(mask)

    def wprep(W):
        return np.ascontiguousarray(
            np.asarray(W, np.float32).T.reshape(NDC, 128, D)
        ).astype(BF)

    wall_a = np.ascontiguousarray(np.concatenate(
        [wprep(Wq), wprep(Wk), wprep(Wv), wprep(Wo)], axis=2))
    bq_a = np.asarray(bq, np.float32).reshape(NDC, 128).T
    bk_a = np.asarray(bk, np.float32).reshape(NDC, 128).T
    bvo_a = np.ascontiguousarray(np.concatenate(
        [np.asarray(bv, np.float32).reshape(1, D),
         np.asarray(bo, np.float32).reshape(1, D)], axis=1))

    kT = key.transpose(0, 2, 1)    # [B, D, S]
    vT = value.transpose(0, 2, 1)
    qT = query.transpose(0, 2, 1)

    in_maps = []
    for core in range(8):
        b, qh = core // 2, core % 2
        xq_a = qT[b][:, qh * SQ:(qh + 1) * SQ].reshape(NDC, 128, SQ)
        xk_a = kT[b].reshape(NDC, 128, S)
        xv_a = vT[b].reshape(NDC, 128, S)
        xall_a = np.ascontiguousarray(
            np.concatenate([xq_a, xk_a, xv_a], axis=2)).astype(BF)
        mb = np.where(mask[b, 0] == 0, np.float32(-1e9), np.float32(0.0))
        mb = mb.reshape(NKT, 128).T
        ball_a = np.ascontiguousarray(
            np.concatenate([bq_a, bk_a, mb], axis=1)).astype(np.float32)
        in_maps.append({
            "xall": xall_a, "wall": wall_a, "ball": ball_a, "bvo": bvo_a,
        })
    return in_maps


def assemble_output(results):
    full = np.empty((B, S, D), np.float32)
    for core in range(8):
        b, qh = core // 2, core % 2
        full[b, qh * SQ:(qh + 1) * SQ, :] = results[core]["out"]
    return full


def kernel(**inputs):
    nc = build_nc()
    in_maps = make_in_maps(**inputs)
    res = run_bass_kernel_spmd(nc, in_maps, list(range(8))).results
    return assemble_output(res)
